# revision 4
# baseline (speedup 1.0000x reference)
"""Epipolar attention kernel for Trainium2 (8 NeuronCores, batch-parallel).

Host does the O(B) 3x3 geometry (SVD etc.) plus the per-row logit max in
float32 numpy; the device does all O(N^2) / O(N^2*C) work:
  d5[i,j]   = |5*(A_j*x_i + B_j*y_i + C_j)|        (PE, exact triple-bf16 split)
  e[i,j]    = exp(d5 - m_i), r_i = rowsum          (ACT, fused accum; m_i from host)
  E2[i,j]   = exp(-e/r)                            (ACT, per-partition scale)
  attnT     = E2^T / colsum(E2)                    (DMA xbar transpose + DVE scale)
  out[i,c]  = sum_j attnT[j,i] * fsrcT[j,c]        (PE, fp16, ldweights reuse)
The double softmax identity: softmax_i(1 - p) == softmax_i(-p) == E2/colsum.
"""

import numpy as np
import ml_dtypes

import concourse.bass as bass
import concourse.bacc as bacc
import concourse.tile as tile
from concourse import mybir
from concourse.bass_utils import run_bass_kernel_spmd

B, C, H, W = 8, 1152, 32, 32
N = H * W           # 1024
P = 128
NT = N // P         # 8
F32 = mybir.dt.float32
F16 = mybir.dt.float16
BF16 = mybir.dt.bfloat16
BFNP = ml_dtypes.bfloat16

TRACE = False
LAST_RESULTS = None
REUSE_WEIGHTS = True   # elide LDWEIGHTS for back-to-back matmuls w/ same lhsT
DMA_TRANSPOSE = True   # use xbar DMA transpose instead of PE transpose

CCH = ((0, 512), (512, 512), (1024, 128))


# ----------------------------------------------------------------- device ---

def _build_nc():
    nc = bacc.Bacc()
    fsrcT = nc.dram_tensor("fsrcT", (N, C), F16, kind="ExternalInput")
    abc9 = nc.dram_tensor("abc9", (9, N), BF16, kind="ExternalInput")
    xy9 = nc.dram_tensor("xy9", (9, N), BF16, kind="ExternalInput")
    # aux[:, 0:NT] = -(rowmax+1) per (p, it); aux[:, NT] = -1.0
    aux = nc.dram_tensor("aux", (P, NT + 1), F32, kind="ExternalInput")
    out = nc.dram_tensor("out", (N, C), F16, kind="ExternalOutput")

    AF = mybir.ActivationFunctionType
    AO = mybir.AluOpType
    I32 = mybir.dt.int32

    with tile.TileContext(nc) as tc:
        with (
            tc.tile_pool(name="consts", bufs=1) as consts,
            tc.tile_pool(name="persist", bufs=1) as persist,
            tc.tile_pool(name="work", bufs=2) as work,
            tc.tile_pool(name="osb", bufs=2) as osbp,
            tc.tile_pool(name="stats", bufs=8) as stats,
            tc.tile_pool(name="psA", bufs=1, space="PSUM") as psA,
            tc.tile_pool(name="psC", bufs=2, space="PSUM") as psC,
        ):
            xy_sb = consts.tile([9, N], BF16, tag="xy")
            nc.sync.dma_start(out=xy_sb, in_=xy9[:, :])
            abc_sb = consts.tile([9, N], BF16, tag="abc")
            nc.sync.dma_start(out=abc_sb, in_=abc9[:, :])
            aux_sb = consts.tile([P, NT + 1], F32, tag="aux")
            nc.sync.dma_start(out=aux_sb, in_=aux[:, :])

            fs_sb = persist.tile([P, NT, C], F16, tag="fs")
            fsv = fsrcT.rearrange("(j p) c -> p j c", p=P)
            for h in range(2):
                nc.sync.dma_start(
                    out=fs_sb[:, h * 4:(h + 1) * 4, :],
                    in_=fsv[:, h * 4:(h + 1) * 4, :],
                )
            e2_sb = persist.tile([P, NT, N], F16, tag="e2")
            at_sb = persist.tile([P, NT, N], F16, tag="at")

            # Phase A: rows i on partitions, j on free dim
            for it in range(NT):
                d_ps = psA.tile([P, N], F32)
                for h in range(2):
                    mm = nc.tensor.matmul(
                        d_ps[:, h * 512:(h + 1) * 512],
                        lhsT=xy_sb[:, it * P:(it + 1) * P],
                        rhs=abc_sb[:, h * 512:(h + 1) * 512],
                        start=True, stop=True,
                    )
                    if REUSE_WEIGHTS and h > 0:
                        mm.ins.ldweights = False
                dabs = work.tile([P, N], F32, tag="dabs")
                nc.vector.tensor_scalar(
                    out=dabs.bitcast(I32), in0=d_ps.bitcast(I32),
                    scalar1=0x7FFFFFFF, scalar2=None, op0=AO.bitwise_and,
                )
                e_t = work.tile([P, N], F32, tag="e")
                r = stats.tile([P, 1], F32, tag="r")
                nc.scalar.activation(
                    out=e_t, in_=dabs, func=AF.Exp,
                    bias=aux_sb[:, it:it + 1], scale=1.0, accum_out=r,
                )
                negr = stats.tile([P, 1], F32, tag="negr")
                nc.vector.tensor_scalar_mul(negr, r, -1.0)
                ninvr = stats.tile([P, 1], F32, tag="ninvr")
                nc.vector.reciprocal(ninvr, negr)
                nc.scalar.activation(
                    out=e2_sb[:, it, :], in_=e_t, func=AF.Exp, bias=0.0,
                    scale=ninvr,
                )
                # xbar transpose: at[q, u, it*P+p] = e2[p, it, u*P+q]
                nc.sync.dma_start_transpose(
                    out=at_sb[:, :, it * P:(it + 1) * P],
                    in_=e2_sb[:, it, :],
                )

            # Phase B: column sums + scale attnT rows in place
            for u in range(NT):
                S = stats.tile([P, 1], F32, tag="S")
                nc.vector.tensor_reduce(
                    out=S, in_=at_sb[:, u, :], axis=mybir.AxisListType.X,
                    op=AO.add,
                )
                invS = stats.tile([P, 1], F32, tag="invS")
                nc.vector.reciprocal(invS, S)
                nc.vector.tensor_scalar_mul(at_sb[:, u, :], at_sb[:, u, :], invS)

            # Phase C: out[i,c] = sum_j attnT[j,i] * fsrcT[j,c]
            for it in range(NT):
                oc = psC.tile([P, 1536], F32, tag="oc")
                for j in range(NT):
                    for ck, (c0, cw) in enumerate(CCH):
                        mm = nc.tensor.matmul(
                            oc[:, ck * 512:ck * 512 + cw],
                            lhsT=at_sb[:, j, it * P:(it + 1) * P],
                            rhs=fs_sb[:, j, c0:c0 + cw],
                            start=(j == 0), stop=(j == NT - 1),
                        )
                        if REUSE_WEIGHTS and ck > 0:
                            mm.ins.ldweights = False
                osb = osbp.tile([P, C], F16, tag="osb")
                nc.scalar.copy(osb[:, 0:512], oc[:, 0:512])
                nc.vector.tensor_copy(osb[:, 512:1024], oc[:, 512:1024])
                nc.scalar.copy(osb[:, 1024:1152], oc[:, 1024:1152])
                nc.sync.dma_start(
                    out=out[it * P:(it + 1) * P, :], in_=osb[:, :]
                )
    nc.compile()
    return nc


_NC = None


def _get_nc():
    global _NC
    if _NC is None:
        _NC = _build_nc()
    return _NC


# ------------------------------------------------------------------- host ---

def _skew(t):
    z = np.zeros_like(t[:, 0])
    return np.stack([
        np.stack([z, -t[:, 2], t[:, 1]], -1),
        np.stack([t[:, 2], z, -t[:, 0]], -1),
        np.stack([-t[:, 1], t[:, 0], z], -1),
    ], 1)


def _fundamental(K1, K2, R, t):
    E = _skew(t) @ R
    U, S, Vt = np.linalg.svd(E)
    S = S.copy()
    S[:, 2] = 0.0
    E = U @ (S[:, :, None] * Vt)
    return np.linalg.inv(np.swapaxes(K2, 1, 2)) @ E @ np.linalg.inv(K1)


def _split3(v):
    """Exact-ish triple bf16 split: v ~= hi + mid + lo (24 mantissa bits)."""
    v = v.astype(np.float32)
    hi = v.astype(BFNP)
    r1 = v - hi.astype(np.float32)
    mid = r1.astype(BFNP)
    r2 = r1 - mid.astype(np.float32)
    lo = r2.astype(BFNP)
    return hi, mid, lo


def _host_prep(f_src, K1, K2, R, t):
    ix, iy = np.meshgrid(np.arange(H, dtype=np.float32),
                         np.arange(W, dtype=np.float32), indexing="ij")
    comb = np.stack([ix.ravel(), iy.ravel(), np.ones(N, np.float32)], 0)  # (3,N)

    F = _fundamental(K1, K2, R, t)                    # (B,3,3)
    lines = (F @ comb).astype(np.float32)             # (B,3,N)
    lines = lines / lines[:, 2:3, :]
    y0 = -lines[:, 2, :] / lines[:, 1, :]
    y1 = -(lines[:, 2, :] + lines[:, 0, :] * np.float32(W)) / lines[:, 1, :]
    dy = y0 - y1
    L = np.sqrt(np.float32(W * W) + dy * dy)
    A5 = np.float32(5.0) * (dy / L)
    B5 = np.float32(5.0) * (np.float32(W) / L)
    C5 = np.float32(-5.0) * (np.float32(W) * y0 / L)

    Ah, Am, Al = _split3(A5)
    Bh, Bm, Bl = _split3(B5)
    Ch, Cm, Cl = _split3(C5)
    abc9 = np.stack([Ah, Bh, Ch, Am, Bm, Cm, Al, Bl, Cl], axis=1)  # (B,9,N) bf16
    xy9 = np.tile(comb, (3, 1)).astype(BFNP)                        # (9,N) exact

    # per-row max of |5*(A_j x_i + B_j y_i + C_j)| over j, plus slack
    d_all = np.einsum(
        "ki,bkj->bij", comb, np.stack([A5, B5, C5], axis=1),
        optimize=True)                                              # (B,N,N)
    m = np.abs(d_all).max(axis=2) + np.float32(1.0)                 # (B,N)
    aux = np.empty((B, P, NT + 1), np.float32)
    aux[:, :, :NT] = -m.reshape(B, NT, P).transpose(0, 2, 1)
    aux[:, :, NT] = -1.0

    fsT = np.ascontiguousarray(
        f_src.reshape(B, C, N).transpose(0, 2, 1)).astype(np.float16)  # (B,N,C)
    return abc9, xy9, aux, fsT


def kernel(f_tar=None, f_src=None, K1=None, K2=None, R=None, t=None):
    global LAST_RESULTS
    f_src = np.asarray(f_src, np.float32)
    K1 = np.asarray(K1, np.float32)
    K2 = np.asarray(K2, np.float32)
    R = np.asarray(R, np.float32)
    t = np.asarray(t, np.float32)

    abc9, xy9, aux, fsT = _host_prep(f_src, K1, K2, R, t)
    in_maps = [
        {"fsrcT": fsT[b], "abc9": np.ascontiguousarray(abc9[b]), "xy9": xy9,
         "aux": np.ascontiguousarray(aux[b])}
        for b in range(B)
    ]
    res = run_bass_kernel_spmd(_get_nc(), in_maps, list(range(B)), trace=TRACE)
    LAST_RESULTS = res
    outs = np.stack([res.results[b]["out"] for b in range(B)], 0)  # (B,N,C) f16
    return outs.astype(np.float32).reshape(B, C, H, W)


# revision 18
# speedup vs baseline: 1.0734x; 1.0734x over previous
"""Epipolar attention kernel for Trainium2 (8 NeuronCores, batch-parallel).

Host does the O(B) 3x3 geometry (SVD etc.) plus the per-row logit max in
float32 numpy; the device does all O(N^2) / O(N^2*C) work:
  d5[i,j]   = |5*(A_j*x_i + B_j*y_i + C_j)|        (PE, exact triple-bf16 split)
  e[i,j]    = exp(d5 - m_i), r_i = rowsum          (ACT, fused accum; m_i from host)
  E2[i,j]   = exp(-e/r)                            (ACT, per-partition scale)
  attnT     = E2^T / colsum(E2)                    (DMA xbar transpose + DVE scale)
  out[i,c]  = sum_j attnT[j,i] * fsrcT[j,c]        (PE, fp16, ldweights reuse)
The double softmax identity: softmax_i(1 - p) == softmax_i(-p) == E2/colsum.
"""

import numpy as np
import ml_dtypes

import concourse.bass as bass
import concourse.bacc as bacc
import concourse.tile as tile
from concourse import mybir
from concourse.bass_utils import run_bass_kernel_spmd

B, C, H, W = 8, 1152, 32, 32
N = H * W           # 1024
P = 128
NT = N // P         # 8
F32 = mybir.dt.float32
F16 = mybir.dt.float16
BF16 = mybir.dt.bfloat16
FP8 = mybir.dt.float8e4
BFNP = ml_dtypes.bfloat16
F8NP = ml_dtypes.float8_e4m3

TRACE = False
LAST_RESULTS = None
REUSE_WEIGHTS = True   # elide LDWEIGHTS for back-to-back matmuls w/ same lhsT
DMA_TRANSPOSE = True   # use xbar DMA transpose instead of PE transpose

CCH = ((0, 512), (512, 512), (1024, 128))


# ----------------------------------------------------------------- device ---

def _elide_redundant_ldweights(m):
    """Drop InstLdweights that reload the exact weights already resident in
    the PE array (same AP, only non-self-loading matmuls in between)."""
    for f in m.functions:
        for bb in f.blocks:
            insts = list(bb.instructions)
            keep = []
            sig = None
            removed = 0
            for i in insts:
                tn = type(i).__name__
                if tn == "InstLdweights":
                    try:
                        s = i.ins[0].concise()
                    except Exception:
                        s = None
                    if (s is not None and s == sig
                            and not i.has_wait() and not i.has_update()):
                        removed += 1
                        continue
                    sig = s
                elif tn == "InstMatmult":
                    if i.ldweights is not False:
                        sig = None
                keep.append(i)
            if removed:
                bb.instructions = keep


def _build_nc():
    nc = bacc.Bacc()
    fsrcT = nc.dram_tensor("fsrcT", (N, C), F16, kind="ExternalInput")
    fsrc8 = nc.dram_tensor("fsrc8", (N, C), FP8, kind="ExternalInput")
    geom = nc.dram_tensor("geom", (9, 2 * N), BF16, kind="ExternalInput")
    # aux[:, 0:NT] = -(rowmax+1) per (p, it); aux[:, NT] = -1.0
    aux = nc.dram_tensor("aux", (P, NT + 1), F32, kind="ExternalInput")
    out = nc.dram_tensor("out", (N, C), F16, kind="ExternalOutput")

    AF = mybir.ActivationFunctionType
    AO = mybir.AluOpType
    I32 = mybir.dt.int32

    with tile.TileContext(nc) as tc:
        with (
            tc.tile_pool(name="consts", bufs=1) as consts,
            tc.tile_pool(name="persist", bufs=1) as persist,
            tc.tile_pool(name="work", bufs=2) as work,
            tc.tile_pool(name="osb", bufs=2) as osbp,
            tc.tile_pool(name="stats", bufs=8) as stats,
            tc.tile_pool(name="ps", bufs=2, space="PSUM") as psp,
        ):
            # prefetch the Exp activation table before any real work
            dmy = stats.tile([P, 1], F32, tag="dmy")
            nc.gpsimd.memset(dmy, 0.0)
            dmy2 = stats.tile([P, 1], F32, tag="dmy2")
            nc.scalar.activation(out=dmy2, in_=dmy, func=AF.Exp, bias=0.0,
                                 scale=1.0)

            geom_sb = consts.tile([9, 2 * N], BF16, tag="geom")
            nc.sync.dma_start(out=geom_sb, in_=geom[:, :])
            xy_sb = geom_sb[:, 0:N]
            abc_sb = geom_sb[:, N:2 * N]
            aux_sb = consts.tile([P, NT + 1], F32, tag="aux")
            nc.sync.dma_start(out=aux_sb, in_=aux[:, :])

            fs_sb = persist.tile([P, NT, C], F16, tag="fs")
            fsv = fsrcT.rearrange("(j p) c -> p j c", p=P)
            for h in range(2):
                nc.sync.dma_start(
                    out=fs_sb[:, h * 4:(h + 1) * 4, :],
                    in_=fsv[:, h * 4:(h + 1) * 4, :],
                )
            fs8_sb = persist.tile([P, NT, C], FP8, tag="fs8")
            fs8v = fsrc8.rearrange("(j p) c -> p j c", p=P)
            nc.scalar.dma_start(out=fs8_sb[:, :, :], in_=fs8v[:, :, :])
            ones_sb = consts.tile([P, P], F16, tag="ones")
            nc.gpsimd.memset(ones_sb, 1.0)
            e2_sb = persist.tile([P, NT, N], F16, tag="e2")
            at_sb = persist.tile([P, NT, N], F16, tag="at")
            at8_sb = persist.tile([P, NT, N], FP8, tag="at8")
            sacc = consts.tile([P, NT], F32, tag="sacc")
            kb_sb = persist.tile([P, C], F16, tag="kb")

            # Phase A: rows i on partitions, j on free dim
            for it in range(NT):
                d_full = psp.tile([P, 1536], F32, tag="ps")
                d_ps = d_full[:, 0:N]
                for h in range(2):
                    mm = nc.tensor.matmul(
                        d_ps[:, h * 512:(h + 1) * 512],
                        lhsT=xy_sb[:, it * P:(it + 1) * P],
                        rhs=abc_sb[:, h * 512:(h + 1) * 512],
                        start=True, stop=True,
                    )
                    if REUSE_WEIGHTS and h > 0:
                        mm.ins.ldweights = False
                dabs = work.tile([P, N], F32, tag="dabs")
                nc.vector.tensor_scalar(
                    out=dabs.bitcast(I32), in0=d_ps.bitcast(I32),
                    scalar1=0x7FFFFFFF, scalar2=None, op0=AO.bitwise_and,
                )
                e_t = work.tile([P, N], F32, tag="e")
                r = stats.tile([P, 1], F32, tag="r")
                nc.scalar.activation(
                    out=e_t, in_=dabs, func=AF.Exp,
                    bias=aux_sb[:, it:it + 1], scale=1.0, accum_out=r,
                )
                negr = stats.tile([P, 1], F32, tag="negr")
                nc.vector.tensor_scalar_mul(negr, r, -1.0)
                ninvr = stats.tile([P, 1], F32, tag="ninvr")
                nc.vector.reciprocal(ninvr, negr)
                nc.scalar.activation(
                    out=e2_sb[:, it, :], in_=e_t, func=AF.Exp, bias=0.0,
                    scale=ninvr,
                )
                # xbar transpose: at[q, u, it*P+p] = e2[p, it, u*P+q]
                nc.sync.dma_start_transpose(
                    out=at_sb[:, :, it * P:(it + 1) * P],
                    in_=e2_sb[:, it, :],
                )
                if it == 0:
                    nc.vector.tensor_reduce(
                        out=sacc, in_=at_sb[:, :, 0:P],
                        axis=mybir.AxisListType.X, op=AO.add,
                    )
                else:
                    part = stats.tile([P, NT], F32, tag="part")
                    nc.vector.tensor_reduce(
                        out=part, in_=at_sb[:, :, it * P:(it + 1) * P],
                        axis=mybir.AxisListType.X, op=AO.add,
                    )
                    nc.vector.tensor_tensor(
                        out=sacc, in0=sacc, in1=part, op=AO.add,
                    )

            # Phase B: invS from accumulated colsums; at8 = (E2^T - 1) * invS
            invS = consts.tile([P, NT], F32, tag="invS")
            nc.vector.reciprocal(invS, sacc)
            for u in range(NT):
                nc.vector.tensor_scalar(
                    out=at8_sb[:, u, :], in0=at_sb[:, u, :],
                    scalar1=-1.0, scalar2=invS[:, u:u + 1],
                    op0=AO.add, op1=AO.mult,
                )
            repl_all = persist.tile([P, NT, P], F16, tag="repl")
            for u in range(NT):
                nc.vector.tensor_scalar_mul(
                    repl_all[:, u, :], ones_sb, invS[:, u:u + 1])

            # Phase C: out[i,c] = Kb[c] + sum_j at8[j,i] * fs8[j,c]  (DoubleRow)
            DR = mybir.MatmulPerfMode.DoubleRow
            kb_done = False
            for it in range(NT):
                oc = psp.tile([P, 1536], F32, tag="ps")
                for pr in range(NT // 2):
                    for ck, (c0, cw) in enumerate(CCH):
                        mm = nc.tensor.matmul(
                            oc[:, ck * 512:ck * 512 + cw],
                            lhsT=at8_sb[:, 2 * pr:2 * pr + 2,
                                        it * P:(it + 1) * P],
                            rhs=fs8_sb[:, 2 * pr:2 * pr + 2, c0:c0 + cw],
                            start=(pr == 0), stop=(pr == NT // 2 - 1),
                            perf_mode=DR,
                        )
                        if REUSE_WEIGHTS and ck > 0:
                            mm.ins.ldweights = False
                if not kb_done:
                    # Kb[q,c] = sum_u sum_p invS_u[p] * fs[u*128+p, c]
                    kb_ps = psp.tile([P, 1536], F32, tag="ps")
                    for ck, (c0, cw) in enumerate(CCH):
                        for u in range(NT):
                            nc.tensor.matmul(
                                kb_ps[:, ck * 512:ck * 512 + cw],
                                lhsT=repl_all[:, u, :],
                                rhs=fs_sb[:, u, c0:c0 + cw],
                                start=(u == 0), stop=(u == NT - 1),
                            )
                    nc.scalar.copy(kb_sb[:, 0:512], kb_ps[:, 0:512])
                    nc.scalar.copy(kb_sb[:, 512:1024], kb_ps[:, 512:1024])
                    nc.scalar.copy(kb_sb[:, 1024:1152], kb_ps[:, 1024:1152])
                    kb_done = True
                osb = osbp.tile([P, C], F16, tag="osb")
                nc.vector.tensor_tensor(
                    out=osb[:, 0:512], in0=oc[:, 0:512],
                    in1=kb_sb[:, 0:512], op=AO.add)
                nc.vector.tensor_tensor(
                    out=osb[:, 512:1024], in0=oc[:, 512:1024],
                    in1=kb_sb[:, 512:1024], op=AO.add)
                nc.vector.tensor_tensor(
                    out=osb[:, 1024:1152], in0=oc[:, 1024:1152],
                    in1=kb_sb[:, 1024:1152], op=AO.add)
                nc.sync.dma_start(
                    out=out[it * P:(it + 1) * P, :], in_=osb[:, :]
                )
    nc.compile()
    if REUSE_WEIGHTS:
        _elide_redundant_ldweights(nc.m)
    return nc


_NC = None


def _get_nc():
    global _NC
    if _NC is None:
        _NC = _build_nc()
    return _NC


# ------------------------------------------------------------------- host ---

def _skew(t):
    z = np.zeros_like(t[:, 0])
    return np.stack([
        np.stack([z, -t[:, 2], t[:, 1]], -1),
        np.stack([t[:, 2], z, -t[:, 0]], -1),
        np.stack([-t[:, 1], t[:, 0], z], -1),
    ], 1)


def _fundamental(K1, K2, R, t):
    E = _skew(t) @ R
    U, S, Vt = np.linalg.svd(E)
    S = S.copy()
    S[:, 2] = 0.0
    E = U @ (S[:, :, None] * Vt)
    return np.linalg.inv(np.swapaxes(K2, 1, 2)) @ E @ np.linalg.inv(K1)


def _split3(v):
    """Exact-ish triple bf16 split: v ~= hi + mid + lo (24 mantissa bits)."""
    v = v.astype(np.float32)
    hi = v.astype(BFNP)
    r1 = v - hi.astype(np.float32)
    mid = r1.astype(BFNP)
    r2 = r1 - mid.astype(np.float32)
    lo = r2.astype(BFNP)
    return hi, mid, lo


def _host_prep(f_src, K1, K2, R, t):
    ix, iy = np.meshgrid(np.arange(H, dtype=np.float32),
                         np.arange(W, dtype=np.float32), indexing="ij")
    comb = np.stack([ix.ravel(), iy.ravel(), np.ones(N, np.float32)], 0)  # (3,N)

    F = _fundamental(K1, K2, R, t)                    # (B,3,3)
    lines = (F @ comb).astype(np.float32)             # (B,3,N)
    lines = lines / lines[:, 2:3, :]
    y0 = -lines[:, 2, :] / lines[:, 1, :]
    y1 = -(lines[:, 2, :] + lines[:, 0, :] * np.float32(W)) / lines[:, 1, :]
    dy = y0 - y1
    L = np.sqrt(np.float32(W * W) + dy * dy)
    A5 = np.float32(5.0) * (dy / L)
    B5 = np.float32(5.0) * (np.float32(W) / L)
    C5 = np.float32(-5.0) * (np.float32(W) * y0 / L)

    Ah, Am, Al = _split3(A5)
    Bh, Bm, Bl = _split3(B5)
    Ch, Cm, Cl = _split3(C5)
    abc9 = np.stack([Ah, Bh, Ch, Am, Bm, Cm, Al, Bl, Cl], axis=1)  # (B,9,N) bf16
    xy9 = np.tile(comb, (3, 1)).astype(BFNP)                        # (9,N) exact
    geom = np.concatenate(
        [np.broadcast_to(xy9[None], (B, 9, N)), abc9], axis=2)      # (B,9,2N)

    # per-row max of |5*(A_j x_i + B_j y_i + C_j)| over j, plus slack
    d_all = np.einsum(
        "ki,bkj->bij", comb, np.stack([A5, B5, C5], axis=1),
        optimize=True)                                              # (B,N,N)
    m = np.abs(d_all).max(axis=2) + np.float32(1.0)                 # (B,N)
    aux = np.empty((B, P, NT + 1), np.float32)
    aux[:, :, :NT] = -m.reshape(B, NT, P).transpose(0, 2, 1)
    aux[:, :, NT] = -1.0

    fsT = np.ascontiguousarray(
        f_src.reshape(B, C, N).transpose(0, 2, 1)).astype(np.float16)  # (B,N,C)
    fs8 = fsT.astype(F8NP)
    return geom, aux, fsT, fs8


def kernel(f_tar=None, f_src=None, K1=None, K2=None, R=None, t=None):
    global LAST_RESULTS
    f_src = np.asarray(f_src, np.float32)
    K1 = np.asarray(K1, np.float32)
    K2 = np.asarray(K2, np.float32)
    R = np.asarray(R, np.float32)
    t = np.asarray(t, np.float32)

    geom, aux, fsT, fs8 = _host_prep(f_src, K1, K2, R, t)
    in_maps = [
        {"fsrcT": fsT[b], "fsrc8": fs8[b], "geom": np.ascontiguousarray(geom[b]),
         "aux": np.ascontiguousarray(aux[b])}
        for b in range(B)
    ]
    res = run_bass_kernel_spmd(_get_nc(), in_maps, list(range(B)), trace=TRACE)
    LAST_RESULTS = res
    outs = np.stack([res.results[b]["out"] for b in range(B)], 0)  # (B,N,C) f16
    return outs.astype(np.float32).reshape(B, C, H, W)


# revision 19
# speedup vs baseline: 1.0954x; 1.0205x over previous
"""Epipolar attention kernel for Trainium2 (8 NeuronCores, batch-parallel).

Host does the O(B) 3x3 geometry (SVD etc.) plus the per-row logit max in
float32 numpy; the device does all O(N^2) / O(N^2*C) work:
  d5[i,j]   = |5*(A_j*x_i + B_j*y_i + C_j)|        (PE, exact triple-bf16 split)
  e[i,j]    = exp(d5 - m_i), r_i = rowsum          (ACT, fused accum; m_i from host)
  E2[i,j]   = exp(-e/r)                            (ACT, per-partition scale)
  attnT     = E2^T / colsum(E2)                    (DMA xbar transpose + DVE scale)
  out[i,c]  = sum_j attnT[j,i] * fsrcT[j,c]        (PE, fp16, ldweights reuse)
The double softmax identity: softmax_i(1 - p) == softmax_i(-p) == E2/colsum.
"""

import numpy as np
import ml_dtypes

import concourse.bass as bass
import concourse.bacc as bacc
import concourse.tile as tile
from concourse import mybir
from concourse.bass_utils import run_bass_kernel_spmd

B, C, H, W = 8, 1152, 32, 32
N = H * W           # 1024
P = 128
NT = N // P         # 8
F32 = mybir.dt.float32
F16 = mybir.dt.float16
BF16 = mybir.dt.bfloat16
FP8 = mybir.dt.float8e4
BFNP = ml_dtypes.bfloat16
F8NP = ml_dtypes.float8_e4m3

TRACE = False
LAST_RESULTS = None
REUSE_WEIGHTS = True   # elide LDWEIGHTS for back-to-back matmuls w/ same lhsT
DMA_TRANSPOSE = True   # use xbar DMA transpose instead of PE transpose

CCH = ((0, 512), (512, 512), (1024, 128))


# ----------------------------------------------------------------- device ---

def _elide_redundant_ldweights(m):
    """Drop InstLdweights that reload the exact weights already resident in
    the PE array (same AP, only non-self-loading matmuls in between)."""
    for f in m.functions:
        for bb in f.blocks:
            insts = list(bb.instructions)
            keep = []
            sig = None
            removed = 0
            for i in insts:
                tn = type(i).__name__
                if tn == "InstLdweights":
                    try:
                        s = i.ins[0].concise()
                    except Exception:
                        s = None
                    if (s is not None and s == sig
                            and not i.has_wait() and not i.has_update()):
                        removed += 1
                        continue
                    sig = s
                elif tn == "InstMatmult":
                    if i.ldweights is not False:
                        sig = None
                keep.append(i)
            if removed:
                bb.instructions = keep


def _build_nc():
    nc = bacc.Bacc()
    fsrcT = nc.dram_tensor("fsrcT", (N, C), F16, kind="ExternalInput")
    fsrc8 = nc.dram_tensor("fsrc8", (N, C), FP8, kind="ExternalInput")
    geom = nc.dram_tensor("geom", (9, 2 * N), BF16, kind="ExternalInput")
    # aux[:, 0:NT] = -(rowmax+1) per (p, it); aux[:, NT] = -1.0
    aux = nc.dram_tensor("aux", (P, NT + 1), F32, kind="ExternalInput")
    out = nc.dram_tensor("out", (N, C), F16, kind="ExternalOutput")

    AF = mybir.ActivationFunctionType
    AO = mybir.AluOpType
    I32 = mybir.dt.int32

    with tile.TileContext(nc) as tc:
        with (
            tc.tile_pool(name="consts", bufs=1) as consts,
            tc.tile_pool(name="persist", bufs=1) as persist,
            tc.tile_pool(name="work", bufs=2) as work,
            tc.tile_pool(name="osb", bufs=2) as osbp,
            tc.tile_pool(name="stats", bufs=8) as stats,
            tc.tile_pool(name="ps", bufs=2, space="PSUM") as psp,
        ):
            # prefetch the Exp activation table before any real work
            dmy = stats.tile([P, 1], F32, tag="dmy")
            nc.gpsimd.memset(dmy, 0.0)
            dmy2 = stats.tile([P, 1], F32, tag="dmy2")
            nc.scalar.activation(out=dmy2, in_=dmy, func=AF.Exp, bias=0.0,
                                 scale=1.0)

            geom_sb = consts.tile([9, 2 * N], BF16, tag="geom")
            nc.sync.dma_start(out=geom_sb, in_=geom[:, :])
            xy_sb = geom_sb[:, 0:N]
            abc_sb = geom_sb[:, N:2 * N]
            aux_sb = consts.tile([P, NT + 1], F32, tag="aux")
            nc.sync.dma_start(out=aux_sb, in_=aux[:, :])

            fs_sb = persist.tile([P, NT, C], F16, tag="fs")
            fsv = fsrcT.rearrange("(j p) c -> p j c", p=P)
            for h in range(2):
                nc.sync.dma_start(
                    out=fs_sb[:, h * 4:(h + 1) * 4, :],
                    in_=fsv[:, h * 4:(h + 1) * 4, :],
                )
            fs8_sb = persist.tile([P, NT, C], FP8, tag="fs8")
            fs8v = fsrc8.rearrange("(j p) c -> p j c", p=P)
            nc.scalar.dma_start(out=fs8_sb[:, :, :], in_=fs8v[:, :, :])
            ones_sb = consts.tile([P, P], F16, tag="ones")
            nc.gpsimd.memset(ones_sb, 1.0)
            e2_sb = persist.tile([P, NT, N], F16, tag="e2")
            at_sb = persist.tile([P, NT, N], F16, tag="at")
            at8_sb = persist.tile([P, NT, N], FP8, tag="at8")
            sacc = consts.tile([P, NT], F32, tag="sacc")
            kb_sb = persist.tile([P, C], F16, tag="kb")

            # Phase A: rows i on partitions, j on free dim
            for it in range(NT):
                d_full = psp.tile([P, 1536], F32, tag="ps")
                d_ps = d_full[:, 0:N]
                for h in range(2):
                    mm = nc.tensor.matmul(
                        d_ps[:, h * 512:(h + 1) * 512],
                        lhsT=xy_sb[:, it * P:(it + 1) * P],
                        rhs=abc_sb[:, h * 512:(h + 1) * 512],
                        start=True, stop=True,
                    )
                    if REUSE_WEIGHTS and h > 0:
                        mm.ins.ldweights = False
                dabs = work.tile([P, N], F32, tag="dabs")
                nc.vector.tensor_scalar(
                    out=dabs.bitcast(I32), in0=d_ps.bitcast(I32),
                    scalar1=0x7FFFFFFF, scalar2=None, op0=AO.bitwise_and,
                )
                e_t = work.tile([P, N], F32, tag="e")
                r = stats.tile([P, 1], F32, tag="r")
                nc.scalar.activation(
                    out=e_t, in_=dabs, func=AF.Exp,
                    bias=aux_sb[:, it:it + 1], scale=1.0, accum_out=r,
                )
                negr = stats.tile([P, 1], F32, tag="negr")
                nc.vector.tensor_scalar_mul(negr, r, -1.0)
                ninvr = stats.tile([P, 1], F32, tag="ninvr")
                nc.vector.reciprocal(ninvr, negr)
                nc.scalar.activation(
                    out=e2_sb[:, it, :], in_=e_t, func=AF.Exp, bias=0.0,
                    scale=ninvr,
                )
                # xbar transpose: at[q, u, it*P+p] = e2[p, it, u*P+q]
                nc.sync.dma_start_transpose(
                    out=at_sb[:, :, it * P:(it + 1) * P],
                    in_=e2_sb[:, it, :],
                )
                if it == 0:
                    nc.vector.tensor_reduce(
                        out=sacc, in_=at_sb[:, :, 0:P],
                        axis=mybir.AxisListType.X, op=AO.add,
                    )
                else:
                    part = stats.tile([P, NT], F32, tag="part")
                    nc.vector.tensor_reduce(
                        out=part, in_=at_sb[:, :, it * P:(it + 1) * P],
                        axis=mybir.AxisListType.X, op=AO.add,
                    )
                    nc.vector.tensor_tensor(
                        out=sacc, in0=sacc, in1=part, op=AO.add,
                    )

            # Phase B: invS from accumulated colsums; at8 = (E2^T - 1) * invS
            invS = consts.tile([P, NT], F32, tag="invS")
            nc.vector.reciprocal(invS, sacc)
            nc.vector.tensor_scalar_mul(invS, invS, 512.0)
            for u in range(NT):
                nc.vector.tensor_scalar(
                    out=at8_sb[:, u, :], in0=at_sb[:, u, :],
                    scalar1=-1.0, scalar2=invS[:, u:u + 1],
                    op0=AO.add, op1=AO.mult,
                )
            repl_all = persist.tile([P, NT, P], F16, tag="repl")
            for u in range(NT):
                nc.vector.tensor_scalar_mul(
                    repl_all[:, u, :], ones_sb, invS[:, u:u + 1])

            # Phase C: out[i,c] = Kb[c] + sum_j at8[j,i] * fs8[j,c]  (DoubleRow)
            DR = mybir.MatmulPerfMode.DoubleRow
            kb_done = False
            for it in range(NT):
                oc = psp.tile([P, 1536], F32, tag="ps")
                for pr in range(NT // 2):
                    for ck, (c0, cw) in enumerate(CCH):
                        mm = nc.tensor.matmul(
                            oc[:, ck * 512:ck * 512 + cw],
                            lhsT=at8_sb[:, 2 * pr:2 * pr + 2,
                                        it * P:(it + 1) * P],
                            rhs=fs8_sb[:, 2 * pr:2 * pr + 2, c0:c0 + cw],
                            start=(pr == 0), stop=(pr == NT // 2 - 1),
                            perf_mode=DR,
                        )
                        if REUSE_WEIGHTS and ck > 0:
                            mm.ins.ldweights = False
                if not kb_done:
                    # Kb[q,c] = sum_u sum_p invS_u[p] * fs[u*128+p, c]
                    kb_ps = psp.tile([P, 1536], F32, tag="ps")
                    for ck, (c0, cw) in enumerate(CCH):
                        for u in range(NT):
                            nc.tensor.matmul(
                                kb_ps[:, ck * 512:ck * 512 + cw],
                                lhsT=repl_all[:, u, :],
                                rhs=fs_sb[:, u, c0:c0 + cw],
                                start=(u == 0), stop=(u == NT - 1),
                            )
                    nc.scalar.copy(kb_sb[:, 0:512], kb_ps[:, 0:512])
                    nc.scalar.copy(kb_sb[:, 512:1024], kb_ps[:, 512:1024])
                    nc.scalar.copy(kb_sb[:, 1024:1152], kb_ps[:, 1024:1152])
                    kb_done = True
                osb = osbp.tile([P, C], F16, tag="osb")
                nc.vector.tensor_tensor(
                    out=osb[:, 0:512], in0=oc[:, 0:512],
                    in1=kb_sb[:, 0:512], op=AO.add)
                nc.vector.tensor_tensor(
                    out=osb[:, 512:1024], in0=oc[:, 512:1024],
                    in1=kb_sb[:, 512:1024], op=AO.add)
                nc.vector.tensor_tensor(
                    out=osb[:, 1024:1152], in0=oc[:, 1024:1152],
                    in1=kb_sb[:, 1024:1152], op=AO.add)
                nc.sync.dma_start(
                    out=out[it * P:(it + 1) * P, :], in_=osb[:, :]
                )
    nc.compile()
    if REUSE_WEIGHTS:
        _elide_redundant_ldweights(nc.m)
    return nc


_NC = None


def _get_nc():
    global _NC
    if _NC is None:
        _NC = _build_nc()
    return _NC


# ------------------------------------------------------------------- host ---

def _skew(t):
    z = np.zeros_like(t[:, 0])
    return np.stack([
        np.stack([z, -t[:, 2], t[:, 1]], -1),
        np.stack([t[:, 2], z, -t[:, 0]], -1),
        np.stack([-t[:, 1], t[:, 0], z], -1),
    ], 1)


def _fundamental(K1, K2, R, t):
    E = _skew(t) @ R
    U, S, Vt = np.linalg.svd(E)
    S = S.copy()
    S[:, 2] = 0.0
    E = U @ (S[:, :, None] * Vt)
    return np.linalg.inv(np.swapaxes(K2, 1, 2)) @ E @ np.linalg.inv(K1)


def _split3(v):
    """Exact-ish triple bf16 split: v ~= hi + mid + lo (24 mantissa bits)."""
    v = v.astype(np.float32)
    hi = v.astype(BFNP)
    r1 = v - hi.astype(np.float32)
    mid = r1.astype(BFNP)
    r2 = r1 - mid.astype(np.float32)
    lo = r2.astype(BFNP)
    return hi, mid, lo


def _host_prep(f_src, K1, K2, R, t):
    ix, iy = np.meshgrid(np.arange(H, dtype=np.float32),
                         np.arange(W, dtype=np.float32), indexing="ij")
    comb = np.stack([ix.ravel(), iy.ravel(), np.ones(N, np.float32)], 0)  # (3,N)

    F = _fundamental(K1, K2, R, t)                    # (B,3,3)
    lines = (F @ comb).astype(np.float32)             # (B,3,N)
    lines = lines / lines[:, 2:3, :]
    y0 = -lines[:, 2, :] / lines[:, 1, :]
    y1 = -(lines[:, 2, :] + lines[:, 0, :] * np.float32(W)) / lines[:, 1, :]
    dy = y0 - y1
    L = np.sqrt(np.float32(W * W) + dy * dy)
    A5 = np.float32(5.0) * (dy / L)
    B5 = np.float32(5.0) * (np.float32(W) / L)
    C5 = np.float32(-5.0) * (np.float32(W) * y0 / L)

    Ah, Am, Al = _split3(A5)
    Bh, Bm, Bl = _split3(B5)
    Ch, Cm, Cl = _split3(C5)
    abc9 = np.stack([Ah, Bh, Ch, Am, Bm, Cm, Al, Bl, Cl], axis=1)  # (B,9,N) bf16
    xy9 = np.tile(comb, (3, 1)).astype(BFNP)                        # (9,N) exact
    geom = np.concatenate(
        [np.broadcast_to(xy9[None], (B, 9, N)), abc9], axis=2)      # (B,9,2N)

    # per-row max of |5*(A_j x_i + B_j y_i + C_j)| over j, plus slack
    d_all = np.einsum(
        "ki,bkj->bij", comb, np.stack([A5, B5, C5], axis=1),
        optimize=True)                                              # (B,N,N)
    m = np.abs(d_all).max(axis=2) + np.float32(1.0)                 # (B,N)
    aux = np.empty((B, P, NT + 1), np.float32)
    aux[:, :, :NT] = -m.reshape(B, NT, P).transpose(0, 2, 1)
    aux[:, :, NT] = -1.0

    fsT = np.ascontiguousarray(
        f_src.reshape(B, C, N).transpose(0, 2, 1)).astype(np.float16)  # (B,N,C)
    fs8 = fsT.astype(F8NP)
    return geom, aux, fsT, fs8


def kernel(f_tar=None, f_src=None, K1=None, K2=None, R=None, t=None):
    global LAST_RESULTS
    f_src = np.asarray(f_src, np.float32)
    K1 = np.asarray(K1, np.float32)
    K2 = np.asarray(K2, np.float32)
    R = np.asarray(R, np.float32)
    t = np.asarray(t, np.float32)

    geom, aux, fsT, fs8 = _host_prep(f_src, K1, K2, R, t)
    in_maps = [
        {"fsrcT": fsT[b], "fsrc8": fs8[b], "geom": np.ascontiguousarray(geom[b]),
         "aux": np.ascontiguousarray(aux[b])}
        for b in range(B)
    ]
    res = run_bass_kernel_spmd(_get_nc(), in_maps, list(range(B)), trace=TRACE)
    LAST_RESULTS = res
    outs = np.stack([res.results[b]["out"] for b in range(B)], 0)  # (B,N,C) f16
    return (outs.astype(np.float32) / np.float32(512.0)).reshape(B, C, H, W)
